# revision 7
# baseline (speedup 1.0000x reference)
"""Context-segment scoring kernel for Trainium2 (Bass/Tile).

Computes out[b, n] = sum_e c[b, n, e] * s[b, e] for
c = c_embeds [32, 32, 32, 8, 256] viewed as [B=32, N=8192, E=256] and
s = s_embeds [32, 256].

Sharding: data-parallel over batch — 8 NeuronCores, 4 batches each.
Per core: stream c (32 MiB) through SBUF in 2 MiB groups
([128 partitions x 16 rows x 256]); multiply by the partition-broadcast
segment embedding and reduce over E. The reduce work is split between
the Vector engine (fused affine_mul_reduce rows, written in place) and
the Scalar engine (activation-Copy accum reduces after a wide Vector
multiply), balanced 9:7 so both engines stay near the ~94 us/core HBM
roofline. Measured: 123 us end-to-end, rel err 3e-07.
"""

import numpy as np

import concourse.bacc as bacc
import concourse.bass as bass
import concourse.mybir as mybir
import concourse.tile as tile
from concourse.bass_utils import run_bass_kernel_spmd

B, N, E = 32, 8192, 256
NCORES = 8
B_LOC = B // NCORES          # 4 batches per core
P = 128                      # SBUF partitions
ROWS = 16                    # n-rows per partition per group
GROUP_N = P * ROWS           # 2048 n per group
G = N // GROUP_N             # 4 groups per batch
NGROUPS = (N // GROUP_N) * B_LOC
# Engine balance: FUSED groups run entirely on DVE via affine_mul_reduce
# (one fused multiply+reduce per row); the rest do one wide DVE multiply
# and per-row ScalarE accum reduces. Bresenham-spread the fused groups so
# both engines stay fed throughout.
# Per-group engine plan: 'A' = fused affine_mul_reduce rows on DVE (in-place,
# no product tile); 'S' = one wide DVE multiply, ScalarE reduces the rows.
# GpSimd elementwise is NOT used: it share-locks the DVE SBUF port and was
# measured to slow every concurrent DVE op by ~36%.
PLAN = ["A", "S", "A", "S", "A", "S", "A", "S",
        "A", "S", "A", "S", "A", "S", "A", "A"]

F32 = mybir.dt.float32
BF16 = mybir.dt.bfloat16


def build_body(tc, out_ap, c_ap, s_ap):
    """Trace the per-core Tile program. APs are DRAM access patterns:
    out [B_LOC, N], c [B_LOC, N, E], s [B_LOC, E]."""
    nc = tc.nc
    with (
        tc.tile_pool(name="sload", bufs=1) as sload_pool,
        tc.tile_pool(name="sbc", bufs=B_LOC) as sbc_pool,
        tc.tile_pool(name="cin", bufs=8) as cin_pool,
        tc.tile_pool(name="prod", bufs=2) as prod_pool,
        tc.tile_pool(name="res", bufs=1) as res_pool,
        tc.tile_pool(name="dump", bufs=2) as dump_pool,
    ):
        # Stage all segment embeddings and broadcast each across partitions.
        # The SWDGE (gpsimd) DMA path casts f32 -> bf16 in the DMA datapath,
        # so SBUF holds bf16 and the DVE runs its 2x 16-bit perf mode.
        s_row = sload_pool.tile([1, B_LOC * E], BF16, tag="s_row")
        nc.gpsimd.dma_start(s_row[:, :], s_ap.rearrange("b e -> (b e)").unsqueeze(0))
        s_sb = []
        for b in range(B_LOC):
            sb = sbc_pool.tile([P, E], BF16, tag="s_sb", name=f"s_sb{b}")
            nc.gpsimd.partition_broadcast(sb[:, :], s_row[0:1, b * E:(b + 1) * E])
            s_sb.append(sb)

        # All per-row results accumulate in one SBUF tile; a single DMA
        # stores them at the end so no tiny output packets interleave with
        # (and stall) the 16-engine input stream mid-kernel.
        res_all = res_pool.tile([P, B_LOC, G, ROWS], F32, tag="res")

        for b in range(B_LOC):
            for g in range(G):
                ct = cin_pool.tile([P, ROWS, E], BF16, tag="cin", name="ct")
                src = c_ap[b, g * GROUP_N:(g + 1) * GROUP_N, :].rearrange(
                    "(p j) e -> p j e", j=ROWS
                )
                nc.gpsimd.dma_start(ct[:], src)

                res = res_all[:, b, g, :]
                gi = b * G + g
                if PLAN[gi % len(PLAN)] == "A":
                    # Fused multiply+reduce per row, entirely on DVE. The
                    # product is written back over the input tile (stream-
                    # safe on DVE) so no product tile or extra sems.
                    for j in range(ROWS):
                        nc.vector.affine_mul_reduce(
                            out=ct[:, j, :],
                            accum_out=res[:, j:j + 1],
                            in0=ct[:, j, :],
                            in1=s_sb[b][:, :],
                            scale=1.0,
                            bias=0.0,
                        )
                else:
                    # One wide DVE multiply, then ScalarE reduces the rows.
                    pr = prod_pool.tile([P, ROWS, E], BF16, tag="prod", name="pr")
                    s_bc = s_sb[b][:, :].unsqueeze(1).broadcast_to([P, ROWS, E])
                    nc.vector.tensor_tensor(
                        out=pr[:],
                        in0=ct[:],
                        in1=s_bc,
                        op=mybir.AluOpType.mult,
                    )
                    dump = dump_pool.tile([P, E], BF16, tag="dump", name="dump")
                    for j in range(ROWS):
                        nc.scalar.activation(
                            dump[:, :],
                            pr[:, j, :],
                            mybir.ActivationFunctionType.Copy,
                            bias=0.0,
                            scale=1.0,
                            accum_out=res[:, j:j + 1],
                        )

        dst = out_ap.rearrange("b (g p j) -> p b g j", g=G, p=P, j=ROWS)
        nc.sync.dma_start(dst, res_all[:, :, :, :])


_NC_CACHE = None


def _get_nc():
    global _NC_CACHE
    if _NC_CACHE is None:
        nc = bacc.Bacc(
            "TRN2",
            target_bir_lowering=False,
            debug=False,
            num_devices=NCORES,
        )
        c = nc.dram_tensor("c", [B_LOC, N, E], F32, kind="ExternalInput")
        s = nc.dram_tensor("s", [B_LOC, E], F32, kind="ExternalInput")
        o = nc.dram_tensor("o", [B_LOC, N], F32, kind="ExternalOutput")
        with tile.TileContext(nc) as tc:
            build_body(tc, o.ap(), c.ap(), s.ap())
        nc.compile()
        _NC_CACHE = nc
    return _NC_CACHE


def _run(c_embeds: np.ndarray, s_embeds: np.ndarray, **kwargs):
    c = np.ascontiguousarray(
        np.asarray(c_embeds, dtype=np.float32).reshape(B, N, E)
    )
    s = np.ascontiguousarray(np.asarray(s_embeds, dtype=np.float32))
    nc = _get_nc()
    in_maps = [
        {
            "c": c[k * B_LOC:(k + 1) * B_LOC],
            "s": s[k * B_LOC:(k + 1) * B_LOC],
        }
        for k in range(NCORES)
    ]
    r = run_bass_kernel_spmd(nc, in_maps, core_ids=list(range(NCORES)), **kwargs)
    out = np.concatenate([r.results[k]["o"] for k in range(NCORES)], axis=0)
    return out.astype(np.float32), r


def kernel(c_embeds: np.ndarray, s_embeds: np.ndarray) -> np.ndarray:
    out, _ = _run(c_embeds, s_embeds)
    return out



# revision 8
# speedup vs baseline: 1.1768x; 1.1768x over previous
"""Context-segment scoring kernel for Trainium2 (Bass/Tile).

Computes out[b, n] = sum_e c[b, n, e] * s[b, e] for
c = c_embeds [32, 32, 32, 8, 256] viewed as [B=32, N=8192, E=256] and
s = s_embeds [32, 256].

Sharding: data-parallel over batch — 8 NeuronCores, 4 batches each.
Per core: stream c (32 MiB) through SBUF in 2 MiB groups
([128 partitions x 16 rows x 256]) on the HWDGE path; every row is a
fused DVE affine_mul_reduce (multiply by the partition-broadcast
segment embedding, reduce over E) writing its [P,1] sum into one
resident result tile. A single DMA stores all results at the end, so
no small packets interleave with the 16-SDMA-engine input stream.
"""

import numpy as np

import concourse.bacc as bacc
import concourse.bass as bass
import concourse.mybir as mybir
import concourse.tile as tile
from concourse.bass_utils import run_bass_kernel_spmd

B, N, E = 32, 8192, 256
NCORES = 8
B_LOC = B // NCORES          # 4 batches per core
P = 128                      # SBUF partitions
ROWS = 16                    # n-rows per partition per group
GROUP_N = P * ROWS           # 2048 n per group
G = N // GROUP_N             # 4 groups per batch

F32 = mybir.dt.float32


def build_body(tc, out_ap, c_ap, s_ap):
    """Trace the per-core Tile program. APs are DRAM access patterns:
    out [B_LOC, N], c [B_LOC, N, E], s [B_LOC, E]."""
    nc = tc.nc
    with (
        tc.tile_pool(name="sload", bufs=1) as sload_pool,
        tc.tile_pool(name="sbc", bufs=B_LOC) as sbc_pool,
        tc.tile_pool(name="cin", bufs=10) as cin_pool,
        tc.tile_pool(name="res", bufs=1) as res_pool,
    ):
        # Stage all segment embeddings and broadcast each across partitions.
        s_row = sload_pool.tile([1, B_LOC * E], F32, tag="s_row")
        nc.sync.dma_start(s_row[:, :], s_ap.rearrange("b e -> (b e)").unsqueeze(0))
        s_sb = []
        for b in range(B_LOC):
            sb = sbc_pool.tile([P, E], F32, tag="s_sb", name=f"s_sb{b}")
            nc.gpsimd.partition_broadcast(sb[:, :], s_row[0:1, b * E:(b + 1) * E])
            s_sb.append(sb)

        # All per-row results accumulate in one SBUF tile; a single DMA
        # stores them at the end.
        res_all = res_pool.tile([P, B_LOC, G, ROWS], F32, tag="res")

        for b in range(B_LOC):
            for g in range(G):
                ct = cin_pool.tile([P, ROWS, E], F32, tag="cin", name="ct")
                src = c_ap[b, g * GROUP_N:(g + 1) * GROUP_N, :].rearrange(
                    "(p j) e -> p j e", j=ROWS
                )
                nc.sync.dma_start(ct[:], src)

                # Fused multiply+reduce per row, entirely on DVE. The
                # product is written back over the input tile (stream-safe
                # on DVE) so no product tile, no cross-engine coupling.
                for j in range(ROWS):
                    nc.vector.affine_mul_reduce(
                        out=ct[:, j, :],
                        accum_out=res_all[:, b, g, j:j + 1],
                        in0=ct[:, j, :],
                        in1=s_sb[b][:, :],
                        scale=1.0,
                        bias=0.0,
                    )

        dst = out_ap.rearrange("b (g p j) -> p b g j", g=G, p=P, j=ROWS)
        nc.sync.dma_start(dst, res_all[:, :, :, :])


_NC_CACHE = None


def _get_nc():
    global _NC_CACHE
    if _NC_CACHE is None:
        nc = bacc.Bacc(
            "TRN2",
            target_bir_lowering=False,
            debug=False,
            num_devices=NCORES,
        )
        c = nc.dram_tensor("c", [B_LOC, N, E], F32, kind="ExternalInput")
        s = nc.dram_tensor("s", [B_LOC, E], F32, kind="ExternalInput")
        o = nc.dram_tensor("o", [B_LOC, N], F32, kind="ExternalOutput")
        with tile.TileContext(nc) as tc:
            build_body(tc, o.ap(), c.ap(), s.ap())
        nc.compile()
        _NC_CACHE = nc
    return _NC_CACHE


def _run(c_embeds: np.ndarray, s_embeds: np.ndarray, **kwargs):
    c = np.ascontiguousarray(
        np.asarray(c_embeds, dtype=np.float32).reshape(B, N, E)
    )
    s = np.ascontiguousarray(np.asarray(s_embeds, dtype=np.float32))
    nc = _get_nc()
    in_maps = [
        {
            "c": c[k * B_LOC:(k + 1) * B_LOC],
            "s": s[k * B_LOC:(k + 1) * B_LOC],
        }
        for k in range(NCORES)
    ]
    r = run_bass_kernel_spmd(nc, in_maps, core_ids=list(range(NCORES)), **kwargs)
    out = np.concatenate([r.results[k]["o"] for k in range(NCORES)], axis=0)
    return out.astype(np.float32), r


def kernel(c_embeds: np.ndarray, s_embeds: np.ndarray) -> np.ndarray:
    out, _ = _run(c_embeds, s_embeds)
    return out
